# revision 4
# baseline (speedup 1.0000x reference)
"""Trainium2 Bass kernel for nn_Attention_72404558676364.

Math: reference computes
    pre[l,b,:] = hs_encoder[l,b,:] @ We.T + (hidden @ Wh.T + b_att)[b,:]
    attn[b,l]  = pre[l,b,:] . v
    out        = softmax(attn, axis=l)
Since softmax over l is shift-invariant, the hidden/Wh/b_att term (constant in
l for fixed b) cancels exactly, and the einsum contracts to a single matvec:
    attn[b,l] = hs_encoder[l,b,:] . w_eff   with   w_eff = We.T @ v
So the device only needs one pass over hs_encoder plus a tiny We.T @ v.

Sharding: data-parallel over batch. Core c handles batches [8c, 8c+8).
hs_encoder shards are pre-transposed on the host to [H, Bc*L] so every DMA is
contiguous >=2KB per partition (fp32 cannot use the DMA-transpose xbar, and
strided-AP transposes are ~19x slower).
"""

import numpy as np

import concourse.bass as bass
import concourse.mybir as mybir
import concourse.tile as tile
from concourse.bass_utils import run_bass_kernel_spmd

H = 1024
L = 512
B = 64
NCORES = 8
BC = B // NCORES  # batches per core
P = 128
HC = H // P  # 128-wide chunks of the contraction dim

F32 = mybir.dt.float32

_split_n = 0


def _split_multi_waits(nc):
    """Hoist extra sem waits onto same-engine NOPs.

    The walrus build in this container rejects any instruction carrying more
    than one sync-wait ("Too many sync wait commands"), but Tile emits
    multi-wait instructions whenever one op depends on several producers.
    A NOP on the same engine immediately before the instruction waits
    equivalently (per-engine program order).
    """
    global _split_n
    for fn in nc.m.functions:
        for blk in fn.blocks:
            new_insts = []
            for inst in blk.instructions:
                si = getattr(inst, "sync_info", None)
                if si is not None and si.on_wait and len(si.on_wait) > 1:
                    waits = list(si.on_wait)
                    si.on_wait = waits[:1]
                    for w in waits[1:]:
                        _split_n += 1
                        new_insts.append(
                            mybir.InstNoOp(
                                name=f"I-wsplit-{_split_n}",
                                engine=inst.engine,
                                sync_info=mybir.SyncInfo(
                                    on_wait=[w], on_update=[]
                                ),
                                bass_nofuse=True,
                            )
                        )
                new_insts.append(inst)
            blk.instructions = new_insts


def _build():
    nc = bass.Bass(target_bir_lowering=False)
    hsT = nc.dram_tensor("hsT", [H, BC * L], F32, kind="ExternalInput")
    we = nc.dram_tensor("We", [H, H], F32, kind="ExternalInput")
    v = nc.dram_tensor("v", [P, HC], F32, kind="ExternalInput")
    out = nc.dram_tensor("out", [BC, L], F32, kind="ExternalOutput")

    with tile.TileContext(nc) as tc:
        with (
            tc.tile_pool(name="singles", bufs=1) as singles,
            tc.tile_pool(name="hs", bufs=4) as hs_pool,
            tc.tile_pool(name="psw", bufs=2, space="PSUM") as psw_pool,
            tc.tile_pool(name="pss", bufs=4, space="PSUM") as pss_pool,
        ):
            # ---- load the small operands -------------------------------
            v_sb = singles.tile([P, HC], F32)
            nc.sync.dma_start(out=v_sb[:], in_=v[:])

            # All of We resident: [128, hc, k] with h = hc*128 + p
            we_sb = singles.tile([P, HC, H], F32)
            nc.sync.dma_start(
                out=we_sb[:], in_=we.rearrange("(hc p) k -> p hc k", p=P)
            )

            # ---- w_eff = We.T @ v, laid out as [128, kc] columns -------
            w_cols = singles.tile([P, HC], F32)
            for kc in range(HC):
                pw = psw_pool.tile([P, 1], F32)
                for hc in range(HC):
                    nc.tensor.matmul(
                        pw[:],
                        lhsT=we_sb[:, hc, kc * P : (kc + 1) * P],
                        rhs=v_sb[:, hc : hc + 1],
                        start=(hc == 0),
                        stop=(hc == HC - 1),
                    )
                nc.vector.tensor_copy(out=w_cols[:, kc : kc + 1], in_=pw[:])

            # ---- scores[j, l] = hsT[:, j*L+l] . w_eff ------------------
            scores1p = singles.tile([1, BC * L], F32)
            for j in range(BC):
                ps = pss_pool.tile([1, L], F32)
                for hc in range(HC):
                    t = hs_pool.tile([P, L], F32)
                    nc.sync.dma_start(
                        out=t[:],
                        in_=hsT[hc * P : (hc + 1) * P, j * L : (j + 1) * L],
                    )
                    nc.tensor.matmul(
                        ps[:],
                        lhsT=w_cols[:, hc : hc + 1],
                        rhs=t[:],
                        start=(hc == 0),
                        stop=(hc == HC - 1),
                    )
                nc.scalar.copy(out=scores1p[0:1, j * L : (j + 1) * L], in_=ps[:])

            # ---- softmax over l per batch row --------------------------
            scoresb = singles.tile([BC, L], F32)
            nc.sync.dma_start(out=scoresb[:], in_=scores1p[:])

            negmax = singles.tile([BC, 1], F32)
            nc.vector.reduce_max(
                out=negmax[:], in_=scoresb[:], axis=mybir.AxisListType.X,
                negate=True,
            )
            exps = singles.tile([BC, L], F32)
            sums = singles.tile([BC, 1], F32)
            nc.scalar.activation(
                out=exps[:],
                in_=scoresb[:],
                func=mybir.ActivationFunctionType.Exp,
                bias=negmax[:],
                scale=1.0,
                accum_out=sums[:],
            )
            rsum = singles.tile([BC, 1], F32)
            nc.vector.reciprocal(out=rsum[:], in_=sums[:])
            outb = singles.tile([BC, L], F32)
            nc.vector.tensor_scalar_mul(out=outb[:], in0=exps[:], scalar1=rsum[:])
            nc.sync.dma_start(out=out[:], in_=outb[:])

    _split_multi_waits(nc)
    return nc


_NC_CACHE = None


def kernel(hidden, hs_encoder, W_att, b_att, vector):
    global _NC_CACHE
    if _NC_CACHE is None:
        _NC_CACHE = _build()
    nc = _NC_CACHE

    hs_encoder = np.ascontiguousarray(hs_encoder, dtype=np.float32)
    we_np = np.ascontiguousarray(W_att[:, H:], dtype=np.float32)
    v_np = np.ascontiguousarray(
        np.asarray(vector, dtype=np.float32)[:, 0].reshape(HC, P).T
    )

    in_maps = []
    for c in range(NCORES):
        shard = hs_encoder[:, c * BC : (c + 1) * BC, :]  # [L, BC, H]
        hst = np.ascontiguousarray(shard.transpose(2, 1, 0).reshape(H, BC * L))
        in_maps.append({"hsT": hst, "We": we_np, "v": v_np})

    res = run_bass_kernel_spmd(nc, in_maps, core_ids=list(range(NCORES)))
    out = np.concatenate([res.results[c]["out"] for c in range(NCORES)], axis=0)
    return out[:, None, :].astype(np.float32)


# revision 5
# speedup vs baseline: 1.4807x; 1.4807x over previous
"""Trainium2 Bass kernel for nn_Attention_72404558676364.

Math: reference computes
    pre[l,b,:] = hs_encoder[l,b,:] @ We.T + (hidden @ Wh.T + b_att)[b,:]
    attn[b,l]  = pre[l,b,:] . v
    out        = softmax(attn, axis=l)
Since softmax over l is shift-invariant, the hidden/Wh/b_att term (constant in
l for fixed b) cancels exactly, and the einsum contracts to a single matvec:
    attn[b,l] = hs_encoder[l,b,:] . w_eff   with   w_eff = We.T @ v
So the device only needs one pass over hs_encoder plus a tiny We.T @ v.

Sharding: data-parallel over batch. Core c handles batches [8c, 8c+8).
hs_encoder shards are pre-transposed on the host to [H, Bc*L] so every DMA is
contiguous >=2KB per partition (fp32 cannot use the DMA-transpose xbar, and
strided-AP transposes are ~19x slower).
"""

import numpy as np

import concourse.bass as bass
import concourse.mybir as mybir
import concourse.tile as tile
from concourse.bass_utils import run_bass_kernel_spmd

H = 1024
L = 512
B = 64
NCORES = 8
BC = B // NCORES  # batches per core
P = 128
HC = H // P  # 128-wide chunks of the contraction dim

F32 = mybir.dt.float32

_split_n = 0


def _split_multi_waits(nc):
    """Hoist extra sem waits onto same-engine NOPs.

    The walrus build in this container rejects any instruction carrying more
    than one sync-wait ("Too many sync wait commands"), but Tile emits
    multi-wait instructions whenever one op depends on several producers.
    A NOP on the same engine immediately before the instruction waits
    equivalently (per-engine program order).
    """
    global _split_n
    for fn in nc.m.functions:
        for blk in fn.blocks:
            new_insts = []
            for inst in blk.instructions:
                si = getattr(inst, "sync_info", None)
                if si is not None and si.on_wait and len(si.on_wait) > 1:
                    waits = list(si.on_wait)
                    si.on_wait = waits[:1]
                    for w in waits[1:]:
                        _split_n += 1
                        new_insts.append(
                            mybir.InstNoOp(
                                name=f"I-wsplit-{_split_n}",
                                engine=inst.engine,
                                sync_info=mybir.SyncInfo(
                                    on_wait=[w], on_update=[]
                                ),
                                bass_nofuse=True,
                            )
                        )
                new_insts.append(inst)
            blk.instructions = new_insts


def _build():
    nc = bass.Bass(target_bir_lowering=False)
    hsT = nc.dram_tensor("hsT", [H, BC * L], F32, kind="ExternalInput")
    we = nc.dram_tensor("We", [H, H], F32, kind="ExternalInput")
    v = nc.dram_tensor("v", [P, HC], F32, kind="ExternalInput")
    out = nc.dram_tensor("out", [BC, L], F32, kind="ExternalOutput")

    with tile.TileContext(nc) as tc:
        with (
            tc.tile_pool(name="singles", bufs=1) as singles,
            tc.tile_pool(name="wep", bufs=16) as we_pool,
            tc.tile_pool(name="hs", bufs=24) as hs_pool,
            tc.tile_pool(name="srow", bufs=2) as srow_pool,
            tc.tile_pool(name="psw", bufs=2, space="PSUM") as psw_pool,
            tc.tile_pool(name="pst", bufs=2, space="PSUM") as pst_pool,
            tc.tile_pool(name="pss", bufs=4, space="PSUM") as pss_pool,
        ):
            # ---- small operands ---------------------------------------
            v_sb = singles.tile([P, HC], F32)
            nc.sync.dma_start(out=v_sb[:], in_=v[:])
            ident = singles.tile([1, 1], F32)
            nc.vector.memset(ident[:], 1.0)

            # ---- w_eff = We.T @ v as a [1, H] row ---------------------
            # lhsT = v chunk [128,1] (trivial stationary), rhs = We chunk
            # [128, 512].  Two psum halves accumulate over the 8 h-chunks.
            w_row = singles.tile([1, H], F32)
            for half in range(2):
                ph = psw_pool.tile([1, L], F32)
                for hc in range(HC):
                    wt = we_pool.tile([P, L], F32)
                    nc.sync.dma_start(
                        out=wt[:],
                        in_=we[hc * P : (hc + 1) * P, half * L : (half + 1) * L],
                    )
                    nc.tensor.matmul(
                        ph[:],
                        lhsT=v_sb[:, hc : hc + 1],
                        rhs=wt[:],
                        start=(hc == 0),
                        stop=(hc == HC - 1),
                    )
                nc.scalar.copy(out=w_row[0:1, half * L : (half + 1) * L], in_=ph[:])

            # ---- w_row -> w_cols[p, hc] = w_eff[hc*128+p] -------------
            # PE transpose of each [1,128] slice into a [128,1] psum.
            w_cols = singles.tile([P, HC], F32)
            for hc in range(HC):
                pt = pst_pool.tile([P, 1], F32)
                nc.tensor.transpose(
                    pt[:], w_row[0:1, hc * P : (hc + 1) * P], ident[:]
                )
                nc.vector.tensor_copy(out=w_cols[:, hc : hc + 1], in_=pt[:])

            # ---- scores[j, l] = hsT[:, j*L+l] . w_eff ------------------
            # hsT tiles [128, 1024] span two batches; all DMAs issue early.
            scoresb = singles.tile([BC, L], F32)
            for jp in range(BC // 2):
                tiles = []
                for hc in range(HC):
                    t = hs_pool.tile([P, 2 * L], F32)
                    nc.sync.dma_start(
                        out=t[:],
                        in_=hsT[hc * P : (hc + 1) * P, jp * 2 * L : (jp + 1) * 2 * L],
                    )
                    tiles.append(t)
                for j in (2 * jp, 2 * jp + 1):
                    off = (j % 2) * L
                    ps = pss_pool.tile([1, L], F32)
                    for hc in range(HC):
                        nc.tensor.matmul(
                            ps[:],
                            lhsT=w_cols[:, hc : hc + 1],
                            rhs=tiles[hc][:, off : off + L],
                            start=(hc == 0),
                            stop=(hc == HC - 1),
                        )
                    srow = srow_pool.tile([1, L], F32)
                    nc.scalar.copy(out=srow[:], in_=ps[:])
                    nc.sync.dma_start(out=scoresb[j : j + 1, :], in_=srow[:])

            # ---- softmax over l per batch row --------------------------
            negmax = singles.tile([BC, 1], F32)
            nc.vector.reduce_max(
                out=negmax[:], in_=scoresb[:], axis=mybir.AxisListType.X,
                negate=True,
            )
            exps = singles.tile([BC, L], F32)
            sums = singles.tile([BC, 1], F32)
            nc.scalar.activation(
                out=exps[:],
                in_=scoresb[:],
                func=mybir.ActivationFunctionType.Exp,
                bias=negmax[:],
                scale=1.0,
                accum_out=sums[:],
            )
            rsum = singles.tile([BC, 1], F32)
            nc.vector.reciprocal(out=rsum[:], in_=sums[:])
            outb = singles.tile([BC, L], F32)
            nc.vector.tensor_scalar_mul(out=outb[:], in0=exps[:], scalar1=rsum[:])
            nc.sync.dma_start(out=out[:], in_=outb[:])

    _split_multi_waits(nc)
    return nc


_NC_CACHE = None


def kernel(hidden, hs_encoder, W_att, b_att, vector):
    global _NC_CACHE
    if _NC_CACHE is None:
        _NC_CACHE = _build()
    nc = _NC_CACHE

    hs_encoder = np.ascontiguousarray(hs_encoder, dtype=np.float32)
    we_np = np.ascontiguousarray(W_att[:, H:], dtype=np.float32)
    v_np = np.ascontiguousarray(
        np.asarray(vector, dtype=np.float32)[:, 0].reshape(HC, P).T
    )

    in_maps = []
    for c in range(NCORES):
        shard = hs_encoder[:, c * BC : (c + 1) * BC, :]  # [L, BC, H]
        hst = np.ascontiguousarray(shard.transpose(2, 1, 0).reshape(H, BC * L))
        in_maps.append({"hsT": hst, "We": we_np, "v": v_np})

    res = run_bass_kernel_spmd(nc, in_maps, core_ids=list(range(NCORES)))
    out = np.concatenate([res.results[c]["out"] for c in range(NCORES)], axis=0)
    return out[:, None, :].astype(np.float32)
